# revision 36
# baseline (speedup 1.0000x reference)
"""AffinityFC Trainium2 kernel (Bass/Tile, 8 NeuronCores, data-parallel over B).

Math per batch b (one NeuronCore per batch):
    px = X[b] @ W1x.T          (Nx=128, hd=1024)
    py = Y[b] @ W1y.T          (Ny=128, hd=1024)
    out[n, m] = W2 . relu(px[n, :] + py[m, :] + b1) + b2

Reformulation: with s = px + b1,
    relu(py + s) = max(py, -s) + s
so the device computes u = max(py, -s) and reduces Sum_h W2[h]*u with
TensorE; the Sum_h W2[h]*s[n,h] term is a rank-1 correction gamma[b,n]
added on the host along with b2.

Engine split per h-chunk (32 four-row blocks g; psum bank g//4, col-group
g%4):
  - DVE blocks (DVE_RUNS, 28): tensor_tensor max at 2x bf16 on the rep4
    (m,j)-interleaved layout (both operands innermost step-1).
  - ACT blocks (ACT_NBS, 4): true relu(py + s[n]) on ScalarE, j-major
    contiguous FD=128 ops reading the plain-py chunk; these rows skip the
    host-side gamma correction.
  - layer-1 PSUM evacuation (negs / s-subset / plain py / py rep4) all on
    ScalarE; VectorE runs nothing but the max stream.
  - Second layer: 32 matmuls per chunk cycling the four col-groups within
    a bank back-to-back, which the PE runs concurrently (~4ns stagger per
    col-group, ~130ns/matmul effective).
  - TensorE warmup matmuls on garbage data un-throttle the HAM clock gate
    before layer-1's data arrives.
  - Last chunk drains bank-major (bank 2 first -- its ScalarE tiles finish
    earliest) with a finer-split DVE tail; every bank is evacuated on
    ScalarE (idle by then; keeps the DVE queue clear) and DMA'd out
    immediately so output transfer overlaps the remaining compute.
"""

import numpy as np
import ml_dtypes

import concourse.mybir as mybir
import concourse.tile as tile
from concourse import bacc
from concourse.bass import ts
from concourse.bass_utils import run_bass_kernel_spmd

B, NX, NY, D, HD = 8, 128, 128, 512, 1024
NCORES = 8
NCH = HD // 128      # 8 h-chunks
KT = D // 128        # 4 k-tiles for the layer-1 contraction
NBLK = NX // 4       # 32 n-blocks of 4 rows each
F32 = mybir.dt.float32
BF16 = mybir.dt.bfloat16

# block g -> psum bank g//4, col-group g%4
ACT_NBS = (8, 9, 10, 11)            # ScalarE true-relu blocks, j-major
DVE_RUNS = ((0, 8), (12, 32))
BANK_ORDER = (0, 1, 3, 4, 5, 6, 7, 2)
ACT_N0 = 4 * ACT_NBS[0]             # first true-relu row
ACT_NN = 4 * len(ACT_NBS)           # number of true-relu rows


def _build_nc(do_compile=True):
    nc = bacc.Bacc(
        "TRN2", target_bir_lowering=False, debug=False, num_devices=NCORES
    )

    xyt = nc.dram_tensor("xyt", [128, KT * (NX + NY)], BF16, kind="ExternalInput")
    w1xt = nc.dram_tensor("w1xt", [128, KT * HD], BF16, kind="ExternalInput")
    w1yt = nc.dram_tensor("w1yt", [128, KT * HD], BF16, kind="ExternalInput")
    b1c = nc.dram_tensor("b1c", [128, 2 * NCH], F32, kind="ExternalInput")
    w2c = nc.dram_tensor("w2c", [128, NCH * 128], BF16, kind="ExternalInput")
    out = nc.dram_tensor("out", [1, NBLK * 512], F32, kind="ExternalOutput")

    with tile.TileContext(nc) as tc:
        with (
            tc.tile_pool(name="const", bufs=1) as cp,
            tc.tile_pool(name="tprod", bufs=3) as tp,
            tc.tile_pool(name="zprod", bufs=3) as zp,
        ):
            xyt_sb = cp.tile([128, KT * (NX + NY)], BF16)
            xt_sb = xyt_sb[:, : KT * NX]
            yt_sb = xyt_sb[:, KT * NX :]
            W1GRP = ((0, 1), (1, 2), (2, 4), (4, NCH))
            w1x_g = [
                cp.tile([128, (hi - lo) * KT * 128], BF16, name=f"w1x{lo}")
                for lo, hi in W1GRP
            ]
            w1y_g = [
                cp.tile([128, (hi - lo) * KT * 128], BF16, name=f"w1y{lo}")
                for lo, hi in W1GRP
            ]

            def w1slab(g, c, k):  # lhsT slab for (chunk c, k-tile)
                for (lo, hi), tile_ in zip(W1GRP, g):
                    if lo <= c < hi:
                        off = ((c - lo) * KT + k) * 128
                        return tile_[:, off : off + 128]
                raise AssertionError

            b1_sb = cp.tile([128, 2 * NCH], F32)
            w2_sb = cp.tile([128, NCH * 128], BF16)
            negs_sb = cp.tile([128, HD], BF16)
            s_sb = cp.tile([128, NCH * ACT_NN], F32)  # +s for relu rows only
            s7_sb = cp.tile([128, 16], F32)  # +s rows 112-127, chunk 7 only
            pyp_sb = cp.tile([128, NCH * 128], BF16)  # plain py per chunk
            pyr_sb = cp.tile([128, NCH * 512], BF16)  # py rep4 per chunk
            out_sc = cp.tile([128, 8 * 512], F32)

            # yt first (py runs first), then xt; w1y on the gpsimd SWDGE
            # queue, w1x on scalar; w2 needed ~14us
            nc.sync.dma_start(out=yt_sb[:, :], in_=xyt[:, KT * NX :])
            nc.sync.dma_start(out=xt_sb[:, :], in_=xyt[:, : KT * NX])
            nc.sync.dma_start(out=b1_sb[:, :], in_=b1c[:, :])
            nc.sync.dma_start(out=w2_sb[:, :], in_=w2c[:, :])
            CW = KT * 128
            for gi, (lo, hi) in enumerate(W1GRP):
                nc.gpsimd.dma_start(
                    out=w1y_g[gi][:, :], in_=w1yt[:, lo * CW : hi * CW]
                )
                nc.scalar.dma_start(
                    out=w1x_g[gi][:, :], in_=w1xt[:, lo * CW : hi * CW]
                )

            # ---- PE warmup: keep TensorE busy pre-data so HAM unthrottles
            # before layer-1; garbage results land in obank partitions and
            # are overwritten by the first start=True matmuls.
            dummy_sb = cp.tile([128, 640], BF16, name="dummy")
            nc.vector.memset(dummy_sb[:, :], 0.0)

            # ---- layer 1 per h-chunk (py first: rep4 gates the DVE stream)
            with tc.tile_pool(name="l1ps", bufs=4, space="PSUM") as l1ps:
                warm = l1ps.tile([128, NX], F32, tag="l1")
                for i in range(14):
                    nc.tensor.matmul(
                        warm[:, :],
                        dummy_sb[:, ts(i % 5, 128)],
                        dummy_sb[:, 128 : 128 + 512].rearrange(
                            "p (a b) -> p a b", a=4
                        )[:, i % 4, :],
                        start=True,
                        stop=True,
                        skip_group_check=True,
                    )
                for c in range(NCH):
                    pyp = l1ps.tile([128, NY], F32, tag="l1")
                    for k in range(KT):
                        nc.tensor.matmul(
                            pyp[:, :],
                            w1slab(w1y_g, c, k),
                            yt_sb[:, ts(k, NY)],
                            start=(k == 0),
                            stop=(k == KT - 1),
                        )
                    nc.scalar.activation(
                        out=pyr_sb[:, ts(c, 512)].rearrange(
                            "p (m j) -> p m j", j=4
                        ),
                        in_=pyp[:, :].unsqueeze(2).broadcast_to((128, 128, 4)),
                        func=mybir.ActivationFunctionType.Copy,
                    )
                    nc.scalar.activation(
                        out=pyp_sb[:, ts(c, 128)],
                        in_=pyp[:, :],
                        func=mybir.ActivationFunctionType.Copy,
                    )
                    pxp = l1ps.tile([128, NX], F32, tag="l1")
                    for k in range(KT):
                        nc.tensor.matmul(
                            pxp[:, :],
                            w1slab(w1x_g, c, k),
                            xt_sb[:, ts(k, NX)],
                            start=(k == 0),
                            stop=(k == KT - 1),
                        )
                    nc.scalar.activation(
                        out=negs_sb[:, ts(c, 128)],
                        in_=pxp[:, :],
                        func=mybir.ActivationFunctionType.Identity,
                        bias=b1_sb[:, NCH + c : NCH + c + 1],
                        scale=-1.0,
                    )
                    nc.scalar.activation(
                        out=s_sb[:, ts(c, ACT_NN)],
                        in_=pxp[:, ACT_N0 : ACT_N0 + ACT_NN],
                        func=mybir.ActivationFunctionType.Identity,
                        bias=b1_sb[:, c : c + 1],
                        scale=1.0,
                    )
                    if c == NCH - 1:
                        nc.scalar.activation(
                            out=s7_sb[:, :],
                            in_=pxp[:, 112:128],
                            func=mybir.ActivationFunctionType.Identity,
                            bias=b1_sb[:, c : c + 1],
                            scale=1.0,
                        )

            # ---- main loop, c-outer; psum slivers resident across chunks
            with tc.tile_pool(name="mps", bufs=1, space="PSUM") as mps:
                obanks = [
                    mps.tile([128, 512], F32, name=f"ob{i}", tag=f"ob{i}")
                    for i in range(8)
                ]
                for c in range(NCH):
                    first, last = c == 0, c == NCH - 1
                    pyr_c = pyr_sb[:, ts(c, 512)]
                    pyr3 = pyr_c.rearrange("p (m j) -> p m j", j=4)
                    tslice = {}

                    runs = DVE_RUNS if not last else (
                        ((0, 8), (12, 22), (22, 25), (25, 28))
                    )
                    for gi, (lo, hi) in enumerate(runs):
                        w = hi - lo
                        t = tp.tile(
                            [128, w * 512], BF16, name=f"t{c}_{gi}",
                            tag=f"t{gi}", bufs=3,
                        )
                        in0 = pyr3.unsqueeze(1).broadcast_to((128, w, 128, 4))
                        base = c * 128 + lo * 4
                        in1 = (
                            negs_sb[:, base : base + 4 * w]
                            .rearrange("p (nbs j) -> p nbs j", j=4)
                            .unsqueeze(2)
                            .broadcast_to((128, w, 128, 4))
                        )
                        nc.vector.tensor_tensor(
                            out=t[:, :].rearrange(
                                "p (nbs m j) -> p nbs m j", nbs=w, m=128
                            ),
                            in0=in0,
                            in1=in1,
                            op=mybir.AluOpType.max,
                        )
                        for nbs in range(w):
                            tslice[lo + nbs] = (t, nbs)

                    # ScalarE true-relu blocks, j-major: relu(py + s[n]);
                    # on the last chunk ScalarE also covers bank 7 (blocks
                    # 28-31, interleaved to match chunks 0-6) so the DVE
                    # stream ends earlier -- the host applies a chunks-0-6
                    # partial gamma to those rows.
                    act_blocks = ACT_NBS if not last else (
                        ACT_NBS + (28, 29, 30, 31)
                    )
                    for nb in act_blocks:
                        ta = zp.tile(
                            [128, 512], BF16, name=f"ta{c}_{nb}", tag="ta",
                            bufs=10,
                        )
                        for j in range(4):
                            n = nb * 4 + j
                            if nb in ACT_NBS:
                                oap = ta[:, ts(j, 128)]
                                bias = s_sb[:, c * ACT_NN + n - ACT_N0 :
                                            c * ACT_NN + n - ACT_N0 + 1]
                            else:
                                oap = ta[:, :].rearrange(
                                    "p (m j) -> p m j", j=4
                                )[:, :, j]
                                bias = s7_sb[:, n - 112 : n - 111]
                            nc.scalar.activation(
                                out=oap,
                                in_=pyp_sb[:, ts(c, 128)],
                                func=mybir.ActivationFunctionType.Relu,
                                bias=bias,
                                scale=1.0,
                            )
                        tslice[nb] = (ta, 0)

                    border = BANK_ORDER if not last else (
                        (2, 0, 1, 3, 4, 5, 6, 7)
                    )
                    for bi, bk in enumerate(border):
                        for jc in range(4):
                            g = bk * 4 + jc
                            t, nbs = tslice[g]
                            mm = nc.tensor.matmul(
                                obanks[bk][32 * jc : 32 * jc + 32, :],
                                w2_sb[:, c * 128 + 32 * jc :
                                      c * 128 + 32 * jc + 32],
                                t[:, ts(nbs, 512)],
                                start=first,
                                stop=last,
                                tile_position=(0, 32 * jc),
                                skip_group_check=True,
                            )
                            if bi > 0:
                                mm.ldweights = False
                        if last:
                            src = obanks[bk][:, :]
                            dst = out_sc[:, ts(bk, 512)]
                            nc.scalar.copy(out=dst, in_=src)
                            qeng = (nc.sync, nc.scalar)[bi % 2]
                            qeng.dma_start(
                                out=out[:, bk * 2048 : (bk + 1) * 2048].rearrange(
                                    "o (jc q) -> (o jc) q", jc=4
                                ),
                                in_=out_sc[0:128:32, ts(bk, 512)],
                            )

    if do_compile:
        nc.compile()
    return nc


_NC_CACHE = None


def _get_nc():
    global _NC_CACHE
    if _NC_CACHE is None:
        _NC_CACHE = _build_nc()
    return _NC_CACHE


def prepare_in_maps(X, Y, W1, b1, W2):
    X = np.asarray(X, dtype=np.float32)
    Y = np.asarray(Y, dtype=np.float32)
    W1 = np.asarray(W1, dtype=np.float32)
    b1 = np.asarray(b1, dtype=np.float32)
    W2 = np.asarray(W2, dtype=np.float32)

    bf = ml_dtypes.bfloat16

    def w1_img(Wh):  # (HD, D) -> flat sbuf image (128, KT*HD)
        return np.ascontiguousarray(
            Wh.reshape(NCH, 128, KT, 128).transpose(3, 0, 2, 1).reshape(128, -1)
        ).astype(bf)

    def xy_img(Xb):  # (N, D) -> flat sbuf image (128, KT*N)
        return np.ascontiguousarray(
            Xb.T.reshape(KT, 128, -1).transpose(1, 0, 2).reshape(128, -1)
        ).astype(bf)

    w1xt = w1_img(W1[:, :D])
    w1yt = w1_img(W1[:, D:])
    b1m = b1.reshape(NCH, 128).T                    # (128, NCH)
    b1cm = np.ascontiguousarray(np.hstack([b1m, -b1m]))
    w2cm = np.ascontiguousarray(
        np.repeat(W2.reshape(NCH, 128).T[:, :, None], 128, axis=2).reshape(128, -1)
    ).astype(bf)

    in_maps = []
    for b in range(B):
        in_maps.append(
            {
                "xyt": np.ascontiguousarray(
                    np.hstack([xy_img(X[b]), xy_img(Y[b])])
                ),
                "w1xt": w1xt,
                "w1yt": w1yt,
                "b1c": b1cm,
                "w2c": w2cm,
            }
        )
    return in_maps


def postprocess(raw_outs, X, W1, b1, W2, b2):
    """DVE blocks: raw[g*512 + m*4 + j]; Z blocks: raw[g*512 + j*128 + m].
    Value = Sum_h W2[h]*max(py,-s) for row 4g+j, col m; add
    gamma[b,n] = (W2@W1x)·X[b,n] + W2·b1, then b2."""
    X = np.asarray(X, dtype=np.float32)
    W1 = np.asarray(W1, dtype=np.float32)
    b1 = np.asarray(b1, dtype=np.float32)
    W2 = np.asarray(W2, dtype=np.float32)
    b2 = np.asarray(b2, dtype=np.float32)

    v = W2[0] @ W1[:, :D]                     # (D,)
    gconst = float(W2[0] @ b1)
    H7 = HD - 128                             # rows 112-127: chunks 0-6 only
    vp = W2[0, :H7] @ W1[:H7, :D]
    gconstp = float(W2[0, :H7] @ b1[:H7])
    aset = set(ACT_NBS)
    out = np.empty((B, NX, NY), dtype=np.float32)
    for b in range(B):
        r = raw_outs[b].reshape(NBLK, 512)
        o = np.empty((NX, NY), dtype=np.float32)
        for g in range(NBLK):
            if g in aset:
                o[4 * g : 4 * g + 4] = r[g].reshape(4, 128)
            else:
                o[4 * g : 4 * g + 4] = r[g].reshape(128, 4).T
        gamma = X[b] @ v + gconst                 # (NX,)
        gamma[112:128] = X[b, 112:128] @ vp + gconstp
        for nb in ACT_NBS:                        # true-relu rows, no gamma
            gamma[nb * 4 : (nb + 1) * 4] = 0.0
        out[b] = o + gamma[:, None] + b2[0]
    return out


def kernel(X, Y, W1, b1, W2, b2):
    in_maps = prepare_in_maps(X, Y, W1, b1, W2)
    nc = _get_nc()
    res = run_bass_kernel_spmd(nc, in_maps, core_ids=list(range(NCORES)))
    raw = [res.results[b]["out"].reshape(-1) for b in range(B)]
    return postprocess(raw, X, W1, b1, W2, b2)


if __name__ == "__main__":
    rng = np.random.default_rng(0)
    ins = {
        "X": rng.standard_normal((B, NX, D), dtype=np.float32),
        "Y": rng.standard_normal((B, NY, D), dtype=np.float32),
        "W1": rng.standard_normal((HD, 2 * D), dtype=np.float32) * (2 * D) ** -0.5,
        "b1": rng.standard_normal((HD,), dtype=np.float32) * (2 * D) ** -0.5,
        "W2": rng.standard_normal((1, HD), dtype=np.float32) * HD**-0.5,
        "b2": rng.standard_normal((1,), dtype=np.float32) * HD**-0.5,
    }
    o = kernel(**ins)
    print("kernel out:", o.shape, o.dtype, float(np.abs(o).max()))


# revision 37
# speedup vs baseline: 1.0265x; 1.0265x over previous
"""AffinityFC Trainium2 kernel (Bass/Tile, 8 NeuronCores, data-parallel over B).

Math per batch b (one NeuronCore per batch):
    px = X[b] @ W1x.T          (Nx=128, hd=1024)
    py = Y[b] @ W1y.T          (Ny=128, hd=1024)
    out[n, m] = W2 . relu(px[n, :] + py[m, :] + b1) + b2

Reformulation: with s = px + b1,
    relu(py + s) = max(py, -s) + s
so the device computes u = max(py, -s) and reduces Sum_h W2[h]*u with
TensorE; the Sum_h W2[h]*s[n,h] term is a rank-1 correction gamma[b,n]
added on the host along with b2.

Engine split per h-chunk (32 four-row blocks g; psum bank g//4, col-group
g%4):
  - DVE blocks (DVE_RUNS, 28): tensor_tensor max at 2x bf16 on the rep4
    (m,j)-interleaved layout (both operands innermost step-1).
  - ACT blocks (ACT_NBS, 4): true relu(py + s[n]) on ScalarE, j-major
    contiguous FD=128 ops reading the plain-py chunk; these rows skip the
    host-side gamma correction.
  - layer-1 PSUM evacuation (negs / s-subset / plain py / py rep4) all on
    ScalarE; VectorE runs nothing but the max stream.
  - Second layer: 32 matmuls per chunk cycling the four col-groups within
    a bank back-to-back, which the PE runs concurrently (~4ns stagger per
    col-group, ~130ns/matmul effective).
  - TensorE warmup matmuls on garbage data un-throttle the HAM clock gate
    before layer-1's data arrives.
  - Last chunk drains bank-major (bank 2 first -- its ScalarE tiles finish
    earliest) with a finer-split DVE tail; every bank is evacuated on
    ScalarE (idle by then; keeps the DVE queue clear) and DMA'd out
    immediately so output transfer overlaps the remaining compute.
"""

import numpy as np
import ml_dtypes

import concourse.mybir as mybir
import concourse.tile as tile
from concourse import bacc
from concourse.bass import ts
from concourse.bass_utils import run_bass_kernel_spmd

B, NX, NY, D, HD = 8, 128, 128, 512, 1024
NCORES = 8
NCH = HD // 128      # 8 h-chunks
KT = D // 128        # 4 k-tiles for the layer-1 contraction
NBLK = NX // 4       # 32 n-blocks of 4 rows each
F32 = mybir.dt.float32
BF16 = mybir.dt.bfloat16

# block g -> psum bank g//4, col-group g%4
ACT_NBS = (8, 9, 10, 11)            # ScalarE true-relu blocks, j-major
DVE_RUNS = ((0, 8), (12, 32))
BANK_ORDER = (0, 1, 3, 4, 5, 6, 7, 2)
ACT_N0 = 4 * ACT_NBS[0]             # first true-relu row
ACT_NN = 4 * len(ACT_NBS)           # number of true-relu rows


def _build_nc(do_compile=True):
    nc = bacc.Bacc(
        "TRN2", target_bir_lowering=False, debug=False, num_devices=NCORES
    )

    xyt = nc.dram_tensor("xyt", [128, KT * (NX + NY)], BF16, kind="ExternalInput")
    w1xt = nc.dram_tensor("w1xt", [128, KT * HD], BF16, kind="ExternalInput")
    w1yt = nc.dram_tensor("w1yt", [128, KT * HD], BF16, kind="ExternalInput")
    b1c = nc.dram_tensor("b1c", [128, 2 * NCH], F32, kind="ExternalInput")
    w2c = nc.dram_tensor("w2c", [128, NCH * 128], BF16, kind="ExternalInput")
    out = nc.dram_tensor("out", [1, NBLK * 512], F32, kind="ExternalOutput")

    with tile.TileContext(nc) as tc:
        with (
            tc.tile_pool(name="const", bufs=1) as cp,
            tc.tile_pool(name="tprod", bufs=3) as tp,
            tc.tile_pool(name="zprod", bufs=3) as zp,
        ):
            xyt_sb = cp.tile([128, KT * (NX + NY)], BF16)
            xt_sb = xyt_sb[:, : KT * NX]
            yt_sb = xyt_sb[:, KT * NX :]
            W1GRP = ((0, 1), (1, 2), (2, 4), (4, NCH))
            w1x_g = [
                cp.tile([128, (hi - lo) * KT * 128], BF16, name=f"w1x{lo}")
                for lo, hi in W1GRP
            ]
            w1y_g = [
                cp.tile([128, (hi - lo) * KT * 128], BF16, name=f"w1y{lo}")
                for lo, hi in W1GRP
            ]

            def w1slab(g, c, k):  # lhsT slab for (chunk c, k-tile)
                for (lo, hi), tile_ in zip(W1GRP, g):
                    if lo <= c < hi:
                        off = ((c - lo) * KT + k) * 128
                        return tile_[:, off : off + 128]
                raise AssertionError

            b1_sb = cp.tile([128, 2 * NCH], F32)
            w2_sb = cp.tile([128, NCH * 128], BF16)
            negs_sb = cp.tile([128, HD], BF16)
            s_sb = cp.tile([128, NCH * ACT_NN], F32)  # +s for relu rows only
            pyp_sb = cp.tile([128, NCH * 128], BF16)  # plain py per chunk
            pyr_sb = cp.tile([128, NCH * 512], BF16)  # py rep4 per chunk
            out_sc = cp.tile([128, 8 * 512], F32)

            # yt first (py runs first), then xt; w1y on the gpsimd SWDGE
            # queue, w1x on scalar; w2 needed ~14us
            nc.sync.dma_start(out=yt_sb[:, :], in_=xyt[:, KT * NX :])
            nc.sync.dma_start(out=xt_sb[:, :], in_=xyt[:, : KT * NX])
            nc.sync.dma_start(out=b1_sb[:, :], in_=b1c[:, :])
            nc.sync.dma_start(out=w2_sb[:, :], in_=w2c[:, :])
            CW = KT * 128
            for gi, (lo, hi) in enumerate(W1GRP):
                nc.gpsimd.dma_start(
                    out=w1y_g[gi][:, :], in_=w1yt[:, lo * CW : hi * CW]
                )
                nc.scalar.dma_start(
                    out=w1x_g[gi][:, :], in_=w1xt[:, lo * CW : hi * CW]
                )

            # ---- PE warmup: keep TensorE busy pre-data so HAM unthrottles
            # before layer-1; garbage results land in obank partitions and
            # are overwritten by the first start=True matmuls.
            dummy_sb = cp.tile([128, 640], BF16, name="dummy")
            nc.vector.memset(dummy_sb[:, :], 0.0)

            # ---- layer 1 per h-chunk (py first: rep4 gates the DVE stream)
            with tc.tile_pool(name="l1ps", bufs=4, space="PSUM") as l1ps:
                warm = l1ps.tile([128, NX], F32, tag="l1")
                for i in range(14):
                    nc.tensor.matmul(
                        warm[:, :],
                        dummy_sb[:, ts(i % 5, 128)],
                        dummy_sb[:, 128 : 128 + 512].rearrange(
                            "p (a b) -> p a b", a=4
                        )[:, i % 4, :],
                        start=True,
                        stop=True,
                        skip_group_check=True,
                    )
                for c in range(NCH):
                    pyp = l1ps.tile([128, NY], F32, tag="l1")
                    for k in range(KT):
                        nc.tensor.matmul(
                            pyp[:, :],
                            w1slab(w1y_g, c, k),
                            yt_sb[:, ts(k, NY)],
                            start=(k == 0),
                            stop=(k == KT - 1),
                        )
                    nc.scalar.activation(
                        out=pyr_sb[:, ts(c, 512)].rearrange(
                            "p (m j) -> p m j", j=4
                        ),
                        in_=pyp[:, :].unsqueeze(2).broadcast_to((128, 128, 4)),
                        func=mybir.ActivationFunctionType.Copy,
                    )
                    nc.scalar.activation(
                        out=pyp_sb[:, ts(c, 128)],
                        in_=pyp[:, :],
                        func=mybir.ActivationFunctionType.Copy,
                    )
                    pxp = l1ps.tile([128, NX], F32, tag="l1")
                    for k in range(KT):
                        nc.tensor.matmul(
                            pxp[:, :],
                            w1slab(w1x_g, c, k),
                            xt_sb[:, ts(k, NX)],
                            start=(k == 0),
                            stop=(k == KT - 1),
                        )
                    nc.scalar.activation(
                        out=negs_sb[:, ts(c, 128)],
                        in_=pxp[:, :],
                        func=mybir.ActivationFunctionType.Identity,
                        bias=b1_sb[:, NCH + c : NCH + c + 1],
                        scale=-1.0,
                    )
                    nc.scalar.activation(
                        out=s_sb[:, ts(c, ACT_NN)],
                        in_=pxp[:, ACT_N0 : ACT_N0 + ACT_NN],
                        func=mybir.ActivationFunctionType.Identity,
                        bias=b1_sb[:, c : c + 1],
                        scale=1.0,
                    )

            # ---- main loop, c-outer; psum slivers resident across chunks
            with tc.tile_pool(name="mps", bufs=1, space="PSUM") as mps:
                obanks = [
                    mps.tile([128, 512], F32, name=f"ob{i}", tag=f"ob{i}")
                    for i in range(8)
                ]
                for c in range(NCH):
                    first, last = c == 0, c == NCH - 1
                    pyr_c = pyr_sb[:, ts(c, 512)]
                    pyr3 = pyr_c.rearrange("p (m j) -> p m j", j=4)
                    tslice = {}

                    runs = DVE_RUNS if not last else (
                        ((0, 8), (12, 22), (22, 26), (26, 30), (30, 32))
                    )
                    for gi, (lo, hi) in enumerate(runs):
                        w = hi - lo
                        t = tp.tile(
                            [128, w * 512], BF16, name=f"t{c}_{gi}",
                            tag=f"t{gi}", bufs=3,
                        )
                        in0 = pyr3.unsqueeze(1).broadcast_to((128, w, 128, 4))
                        base = c * 128 + lo * 4
                        in1 = (
                            negs_sb[:, base : base + 4 * w]
                            .rearrange("p (nbs j) -> p nbs j", j=4)
                            .unsqueeze(2)
                            .broadcast_to((128, w, 128, 4))
                        )
                        nc.vector.tensor_tensor(
                            out=t[:, :].rearrange(
                                "p (nbs m j) -> p nbs m j", nbs=w, m=128
                            ),
                            in0=in0,
                            in1=in1,
                            op=mybir.AluOpType.max,
                        )
                        for nbs in range(w):
                            tslice[lo + nbs] = (t, nbs)

                    # ScalarE true-relu blocks, j-major: relu(py + s[n])
                    for nb in ACT_NBS:
                        ta = zp.tile(
                            [128, 512], BF16, name=f"ta{c}_{nb}", tag="ta",
                            bufs=10,
                        )
                        for j in range(4):
                            n = nb * 4 + j
                            nc.scalar.activation(
                                out=ta[:, ts(j, 128)],
                                in_=pyp_sb[:, ts(c, 128)],
                                func=mybir.ActivationFunctionType.Relu,
                                bias=s_sb[:, c * ACT_NN + n - ACT_N0 :
                                          c * ACT_NN + n - ACT_N0 + 1],
                                scale=1.0,
                            )
                        tslice[nb] = (ta, 0)

                    border = BANK_ORDER if not last else (
                        (2, 0, 1, 3, 4, 5, 6, 7)
                    )
                    for bi, bk in enumerate(border):
                        for jc in range(4):
                            g = bk * 4 + jc
                            t, nbs = tslice[g]
                            mm = nc.tensor.matmul(
                                obanks[bk][32 * jc : 32 * jc + 32, :],
                                w2_sb[:, c * 128 + 32 * jc :
                                      c * 128 + 32 * jc + 32],
                                t[:, ts(nbs, 512)],
                                start=first,
                                stop=last,
                                tile_position=(0, 32 * jc),
                                skip_group_check=True,
                            )
                            if bi > 0:
                                mm.ldweights = False
                        if last:
                            src = obanks[bk][:, :]
                            dst = out_sc[:, ts(bk, 512)]
                            nc.scalar.copy(out=dst, in_=src)
                            qeng = (nc.sync, nc.scalar)[bi % 2]
                            qeng.dma_start(
                                out=out[:, bk * 2048 : (bk + 1) * 2048].rearrange(
                                    "o (jc q) -> (o jc) q", jc=4
                                ),
                                in_=out_sc[0:128:32, ts(bk, 512)],
                            )

    if do_compile:
        nc.compile()
    return nc


_NC_CACHE = None


def _get_nc():
    global _NC_CACHE
    if _NC_CACHE is None:
        _NC_CACHE = _build_nc()
    return _NC_CACHE


def prepare_in_maps(X, Y, W1, b1, W2):
    X = np.asarray(X, dtype=np.float32)
    Y = np.asarray(Y, dtype=np.float32)
    W1 = np.asarray(W1, dtype=np.float32)
    b1 = np.asarray(b1, dtype=np.float32)
    W2 = np.asarray(W2, dtype=np.float32)

    bf = ml_dtypes.bfloat16

    def w1_img(Wh):  # (HD, D) -> flat sbuf image (128, KT*HD)
        return np.ascontiguousarray(
            Wh.reshape(NCH, 128, KT, 128).transpose(3, 0, 2, 1).reshape(128, -1)
        ).astype(bf)

    def xy_img(Xb):  # (N, D) -> flat sbuf image (128, KT*N)
        return np.ascontiguousarray(
            Xb.T.reshape(KT, 128, -1).transpose(1, 0, 2).reshape(128, -1)
        ).astype(bf)

    w1xt = w1_img(W1[:, :D])
    w1yt = w1_img(W1[:, D:])
    b1m = b1.reshape(NCH, 128).T                    # (128, NCH)
    b1cm = np.ascontiguousarray(np.hstack([b1m, -b1m]))
    w2cm = np.ascontiguousarray(
        np.repeat(W2.reshape(NCH, 128).T[:, :, None], 128, axis=2).reshape(128, -1)
    ).astype(bf)

    in_maps = []
    for b in range(B):
        in_maps.append(
            {
                "xyt": np.ascontiguousarray(
                    np.hstack([xy_img(X[b]), xy_img(Y[b])])
                ),
                "w1xt": w1xt,
                "w1yt": w1yt,
                "b1c": b1cm,
                "w2c": w2cm,
            }
        )
    return in_maps


def postprocess(raw_outs, X, W1, b1, W2, b2):
    """DVE blocks: raw[g*512 + m*4 + j]; Z blocks: raw[g*512 + j*128 + m].
    Value = Sum_h W2[h]*max(py,-s) for row 4g+j, col m; add
    gamma[b,n] = (W2@W1x)·X[b,n] + W2·b1, then b2."""
    X = np.asarray(X, dtype=np.float32)
    W1 = np.asarray(W1, dtype=np.float32)
    b1 = np.asarray(b1, dtype=np.float32)
    W2 = np.asarray(W2, dtype=np.float32)
    b2 = np.asarray(b2, dtype=np.float32)

    v = W2[0] @ W1[:, :D]                     # (D,)
    gconst = float(W2[0] @ b1)
    aset = set(ACT_NBS)
    out = np.empty((B, NX, NY), dtype=np.float32)
    for b in range(B):
        r = raw_outs[b].reshape(NBLK, 512)
        o = np.empty((NX, NY), dtype=np.float32)
        for g in range(NBLK):
            if g in aset:
                o[4 * g : 4 * g + 4] = r[g].reshape(4, 128)
            else:
                o[4 * g : 4 * g + 4] = r[g].reshape(128, 4).T
        gamma = X[b] @ v + gconst                 # (NX,)
        for nb in ACT_NBS:                        # true-relu rows, no gamma
            gamma[nb * 4 : (nb + 1) * 4] = 0.0
        out[b] = o + gamma[:, None] + b2[0]
    return out


def kernel(X, Y, W1, b1, W2, b2):
    in_maps = prepare_in_maps(X, Y, W1, b1, W2)
    nc = _get_nc()
    res = run_bass_kernel_spmd(nc, in_maps, core_ids=list(range(NCORES)))
    raw = [res.results[b]["out"].reshape(-1) for b in range(B)]
    return postprocess(raw, X, W1, b1, W2, b2)


if __name__ == "__main__":
    rng = np.random.default_rng(0)
    ins = {
        "X": rng.standard_normal((B, NX, D), dtype=np.float32),
        "Y": rng.standard_normal((B, NY, D), dtype=np.float32),
        "W1": rng.standard_normal((HD, 2 * D), dtype=np.float32) * (2 * D) ** -0.5,
        "b1": rng.standard_normal((HD,), dtype=np.float32) * (2 * D) ** -0.5,
        "W2": rng.standard_normal((1, HD), dtype=np.float32) * HD**-0.5,
        "b2": rng.standard_normal((1,), dtype=np.float32) * HD**-0.5,
    }
    o = kernel(**ins)
    print("kernel out:", o.shape, o.dtype, float(np.abs(o).max()))
